# revision 27
# baseline (speedup 1.0000x reference)
"""Liquid Neural Network Trainium2 kernel — segment-parallel scan.

Reference (per batch element, per step, tau=1 case):
    ie_s   = W_comb @ x_s + b_comb            (input path, folded on host)
    h_next = W_hh @ tanh(h) + ie_s            (contractive: ||W_hh||_2 ~ 0.16)
    out_s  = W_out @ tanh(h_next) + b_out

Key idea: the recurrence is strongly contractive (W_hh scaled by 0.01 =>
spectral norm ~0.16), so state influence decays ~0.16x per step.  We split
each core's 4096-step sequence into NSEG=64 independent segments, each
prefixed with `w` warmup steps seeded from h=0 (error ~0.16^w, far below
the 2e-2 tolerance).  All 64 segments advance one step per "round":

  * 2 partition groups: segments 0-31 on SBUF/PSUM partitions 0-63,
    segments 32-63 on partitions 64-127 via block-diagonal stationaries
    diag(W, W) — one matmul advances both groups.
  * 32 column-segments x 32 batch = 1024 moving columns per round
    (2 PSUM banks; two 512-col matmuls per operation, all bf16 with fp32
    PSUM accumulation).
  * Round pipeline: phase-A matmul (x -> ie', written straight into the
    scan PSUM banks with start=True, one round ahead), scan matmul pair
    (accumulate W . th), one 1024-col ScalarE tanh (PSUM -> SBUF bf16,
    bias port adds b_comb), out-projection matmul pair (prev round) +
    DVE copy, y DMA every 4 rounds.  The critical cycle is
    tanh(r-1) -> scan pair -> tanh(r) (~1.85us); phase A and out-proj
    hide in the PE's slack under the tanh.
  * x chunks stream in on the GpSimd SWDGE queue (decoupled from the
    Sync-queue y DMAs); rounds T = 4096/64 + w instead of 4096 steps.

8-way data parallel over batch (32 rows per core), same NEFF on all cores.
"""

import numpy as np

B, I, H = 256, 32, 64
NCORES = 8
BS = B // NCORES                 # 32 batch rows per core
NSEG = 64                        # segments per core (2 groups x 32)
GRP = NSEG // 2                  # column-segments per partition group
GCOL = GRP * BS                  # 1024 moving columns per round
HB = 512                         # half-round: one fp32 PSUM bank
CH = 4                           # rounds of x per DMA chunk

_nc_cache = {}


def _build(T):
    """Per-core Bass program; identical NEFF on all 8 cores."""
    import concourse.bacc as bacc
    import concourse.tile as tile
    from concourse import mybir

    nc = bacc.Bacc(
        "TRN2",
        target_bir_lowering=False,
        debug=False,
        enable_asserts=False,
        num_devices=NCORES,
    )
    f32 = mybir.dt.float32
    bf16 = mybir.dt.bfloat16
    Tanh = mybir.ActivationFunctionType.Tanh

    ncols = T * GCOL
    x_d = nc.dram_tensor("x", [2 * I, ncols], bf16, kind="ExternalInput")
    comb_d = nc.dram_tensor("p_comb", [2 * I, 2 * H], bf16, kind="ExternalInput")
    scan_d = nc.dram_tensor("p_scan", [2 * H, 2 * H], bf16, kind="ExternalInput")
    wout_d = nc.dram_tensor("p_wout", [2 * H, 2], bf16, kind="ExternalInput")
    bcomb_d = nc.dram_tensor("p_bcomb", [2 * H, 1], f32, kind="ExternalInput")
    y_d = nc.dram_tensor("y", [2, T, GCOL], f32, kind="ExternalOutput")

    x_ap = x_d.ap()
    y_ap = y_d.ap()

    with tile.TileContext(nc) as tc:
        with (
            tc.tile_pool(name="consts", bufs=1) as consts,
            tc.tile_pool(name="xpool", bufs=3) as xpool,
            tc.tile_pool(name="thpool", bufs=5) as thpool,
            tc.tile_pool(name="opool", bufs=6) as opool,
            tc.tile_pool(name="psS", bufs=2, space="PSUM") as psS,
            tc.tile_pool(name="psO", bufs=2, space="PSUM") as psO,
        ):
            comb_sb = consts.tile([2 * I, 2 * H], bf16, name="comb_sb")
            nc.sync.dma_start(out=comb_sb, in_=comb_d.ap())
            scan_sb = consts.tile([2 * H, 2 * H], bf16, name="scan_sb")
            nc.sync.dma_start(out=scan_sb, in_=scan_d.ap())
            wout_sb = consts.tile([2 * H, 2], bf16, name="wout_sb")
            nc.sync.dma_start(out=wout_sb, in_=wout_d.ap())
            bcomb_sb = consts.tile([2 * H, 1], f32, name="bcomb_sb")
            nc.sync.dma_start(out=bcomb_sb, in_=bcomb_d.ap())

            # chunk c covers rounds [bounds[c], bounds[c+1])
            bounds = [0]
            while bounds[-1] < T:
                bounds.append(min(T, bounds[-1] + CH))
            nchunks = len(bounds) - 1
            round_chunk = {}
            for c in range(nchunks):
                for r in range(bounds[c], bounds[c + 1]):
                    round_chunk[r] = c

            xtiles = {}

            def emit_chunk(c):
                lo = bounds[c] * GCOL
                hi = bounds[c + 1] * GCOL
                xt = xpool.tile([2 * I, CH * GCOL], bf16, name=f"x_sb_{c}", tag="x")
                # first two chunks ride the (empty) Sync HWDGE queue for a
                # faster cold start; later ones use GpSimd SWDGE so they
                # never queue behind the per-round y DMAs.
                eng = nc.sync if c < 2 else nc.gpsimd
                eng.dma_start(out=xt[:, : hi - lo], in_=x_ap[:, lo:hi])
                xtiles[c] = xt

            ps_tiles = {}

            def emit_phase_a(r, stop=False):
                ps = psS.tile([2 * H, GCOL], f32, name=f"psS_{r}", tag="psS")
                c = round_chunk[r]
                off = (r - bounds[c]) * GCOL
                for h in range(2):
                    nc.tensor.matmul(
                        ps[:, h * HB : (h + 1) * HB],
                        comb_sb,
                        xtiles[c][:, off + h * HB : off + (h + 1) * HB],
                        start=True,
                        stop=stop,
                        skip_group_check=True,
                    )
                ps_tiles[r] = ps

            OB = 4                       # out-rounds per y DMA
            cur_osb = [None]

            def emit_outproj(th_src, r):
                pso = psO.tile([2, GCOL], f32, name=f"psO_{r}", tag="psO")
                for h in range(2):
                    sl = slice(h * HB, (h + 1) * HB)
                    nc.tensor.matmul(
                        pso[:, sl],
                        wout_sb,
                        th_src[:, sl],
                        start=True,
                        stop=True,
                        skip_group_check=True,
                    )
                if cur_osb[0] is None:
                    cur_osb[0] = opool.tile(
                        [2, OB * GCOL], f32, name=f"osb_{r // OB}", tag="o"
                    )
                off = (r % OB) * GCOL
                nc.vector.tensor_copy(out=cur_osb[0][:, off : off + GCOL], in_=pso)
                if r % OB == OB - 1 or r == T - 1:
                    lo = (r // OB) * OB
                    nc.sync.dma_start(
                        out=y_ap[:, lo : r + 1, :],
                        in_=cur_osb[0][:, : (r + 1 - lo) * GCOL],
                    )
                    cur_osb[0] = None

            # --- prologue: pre-warm the PE clock gate (HAM) and load the
            # Tanh table while the first x chunks stream in.  ~32 dummy
            # matmuls on a zeroed scratch tile keep the PE busy >3.4us so
            # real rounds start at 2.4 GHz instead of 1.2 GHz.
            dummy = consts.tile([2 * H, HB], bf16, name="dummy")
            nc.vector.memset(dummy, 0.0)
            wps = psS.tile([2 * H, GCOL], f32, name="wps", tag="psS")
            warm_th = consts.tile([2 * H, 8], bf16, name="warm_th")
            for i in range(32):
                nc.tensor.matmul(
                    wps[:, :HB],
                    dummy[:, : 2 * H],
                    dummy,
                    start=True,
                    stop=True,
                    skip_group_check=True,
                )
                if i == 8:
                    nc.scalar.activation(out=warm_th, in_=wps[:, :8], func=Tanh)

            emit_chunk(0)
            emit_chunk(1)
            emit_phase_a(0, stop=True)

            th_prev = None
            for r in range(T):
                c = round_chunk[r]
                if r == bounds[c] and c + 2 < nchunks:
                    emit_chunk(c + 2)
                ps = ps_tiles.pop(r)
                th_r = thpool.tile([2 * H, GCOL], bf16, name=f"th_{r}", tag="th")
                # critical cycle: tanh(r-1) -> scan pair -> tanh(r).  One
                # 1024-col tanh (both PSUM banks) keeps ACT occupancy low;
                # out-proj and next-round phase A fill the PE during tanh.
                if r >= 1:
                    for h in range(2):
                        sl = slice(h * HB, (h + 1) * HB)
                        nc.tensor.matmul(
                            ps[:, sl],
                            scan_sb,
                            th_prev[:, sl],
                            start=False,
                            stop=True,
                            skip_group_check=True,
                        )
                nc.scalar.activation(out=th_r, in_=ps, func=Tanh, bias=bcomb_sb)
                if r + 1 < T:
                    emit_phase_a(r + 1)
                if r >= 1:
                    emit_outproj(th_prev, r - 1)
                th_prev = th_r
            emit_outproj(th_prev, T - 1)

    nc.compile()
    return nc


def _numpy_fallback(x, W_in, b_in, W_hh, W_ih, bias, tau, W_out, b_out):
    x = np.asarray(x, np.float32)
    nbatch, n_steps, _ = x.shape
    hid = W_hh.shape[0]
    u = x @ np.asarray(W_in, np.float32).T + np.asarray(b_in, np.float32)
    ie = u @ np.asarray(W_ih, np.float32).T
    tau = np.asarray(tau, np.float32)
    bias = np.asarray(bias, np.float32)
    W_hhT = np.asarray(W_hh, np.float32).T
    W_outT = np.asarray(W_out, np.float32).T
    h = np.zeros((nbatch, hid), np.float32)
    out = np.empty((nbatch, n_steps, W_outT.shape[1]), np.float32)
    for s in range(n_steps):
        dhdt = (-h + np.tanh(h) @ W_hhT + ie[:, s] + bias) / tau
        h = h + dhdt
        out[:, s] = np.tanh(h) @ W_outT
    return out + np.asarray(b_out, np.float32)


def kernel(x, W_in, b_in, W_hh, W_ih, bias, tau, W_out, b_out):
    import ml_dtypes

    x = np.asarray(x, np.float32)
    nbatch, n_steps, nin = x.shape
    tau64 = np.asarray(tau, np.float64)
    bscale = 1.0 / tau64                                   # dt=1
    a = 1.0 - bscale
    general = bool(np.any(a != 0.0))

    W_in64 = np.asarray(W_in, np.float64)
    W_ih64 = np.asarray(W_ih, np.float64)
    W_hh64 = np.asarray(W_hh, np.float64)
    b_in64 = np.asarray(b_in, np.float64)
    bias64 = np.asarray(bias, np.float64)

    p_scan = (bscale[:, None] * W_hh64).T.astype(np.float32)        # [H, H] lhsT
    p_comb = (bscale[:, None] * (W_ih64 @ W_in64)).T.astype(np.float32)  # [I, H]
    p_bcomb = (bscale * (W_ih64 @ b_in64 + bias64)).astype(np.float32)
    p_wout = np.asarray(W_out, np.float32).T                        # [H, 1]

    sigma = float(np.linalg.norm(p_scan, 2))
    if (
        general
        or nbatch != B
        or nin != I
        or W_hh.shape[0] != H
        or n_steps % NSEG != 0
        or n_steps // NSEG < 8
        or sigma > 0.7
    ):
        return _numpy_fallback(x, W_in, b_in, W_hh, W_ih, bias, tau, W_out, b_out)

    w = max(3, int(np.ceil(np.log(5e-3) / np.log(max(sigma, 1e-9)))))
    SEG = n_steps // NSEG
    T = SEG + w

    # block-diagonal stationaries: group A on partitions 0-63, B on 64-127
    p_comb2 = np.zeros((2 * I, 2 * H), np.float32)
    p_comb2[:I, :H] = p_comb
    p_comb2[I:, H:] = p_comb
    p_scan2 = np.zeros((2 * H, 2 * H), np.float32)
    p_scan2[:H, :H] = p_scan
    p_scan2[H:, H:] = p_scan
    p_wout2 = np.zeros((2 * H, 2), np.float32)
    p_wout2[:H, 0] = p_wout[:, 0]
    p_wout2[H:, 1] = p_wout[:, 0]
    p_bcomb2 = np.concatenate([p_bcomb, p_bcomb]).reshape(2 * H, 1)
    p_comb2 = p_comb2.astype(ml_dtypes.bfloat16)
    p_scan2 = p_scan2.astype(ml_dtypes.bfloat16)
    p_wout2 = p_wout2.astype(ml_dtypes.bfloat16)

    key = (T,)
    if key not in _nc_cache:
        _nc_cache[key] = _build(T)
    nc = _nc_cache[key]

    # x -> per-core round-major layout [2I, T*GCOL]:
    # partition g*32+i, column r*GCOL + j*BS + b  =  x[core*BS+b, (g*GRP+j)*SEG - w + r, i]
    xp = np.zeros((nbatch, w + n_steps, nin), np.float32)
    xp[:, w:] = x
    win = np.lib.stride_tricks.sliding_window_view(xp, T, axis=1)[:, ::SEG]
    # win: [nbatch, NSEG, I, T]
    in_maps = []
    for c in range(NCORES):
        wc = win[c * BS : (c + 1) * BS]                  # [BS, NSEG, I, T]
        wc = wc.reshape(BS, 2, GRP, nin, T)              # (b, g, j, i, r)
        xdev = np.ascontiguousarray(
            wc.transpose(1, 3, 4, 2, 0).reshape(2 * I, T * GCOL)
        ).astype(ml_dtypes.bfloat16)
        in_maps.append(
            {
                "x": xdev,
                "p_comb": p_comb2,
                "p_scan": p_scan2,
                "p_wout": p_wout2,
                "p_bcomb": p_bcomb2,
            }
        )

    from concourse.bass_utils import run_bass_kernel_spmd

    res = run_bass_kernel_spmd(nc, in_maps, core_ids=list(range(NCORES)))
    kernel.last_results = res

    y = np.empty((nbatch, n_steps, 1), np.float32)
    for c in range(NCORES):
        yr = np.asarray(res.results[c]["y"], np.float32)    # [2, T, GCOL]
        v = yr[:, w : w + SEG, :].reshape(2, SEG, GRP, BS)  # (g, r', j, b)
        v = v.transpose(0, 2, 1, 3).reshape(n_steps, BS)    # s = (g*GRP+j)*SEG + r'
        y[c * BS : (c + 1) * BS, :, 0] = v.T
    y += float(np.asarray(b_out, np.float32).reshape(-1)[0])
    return y


kernel.last_results = None
